# revision 17
# baseline (speedup 1.0000x reference)
"""Distributed multi-head attention for 8 TRN2 NeuronCores.

Problem: x[2,2048,1024] -> QKV proj (w_qkv[3072,1024]) -> 16-head SDPA ->
out proj (w_proj[1024,1024] + b_proj) -> [2,2048,1024].

Sharding: 2 heads per core (head-parallel over all 8 cores; both batches on
every core). Per core:
  Phase A: qT/kT [128(=2 heads x 64d), 4096] and V-natural [4096, 128] from
           x @ w_qkv_shard.T (fp32r matmuls; V via PE transpose of vT).
  Phase B: transposed-score attention per (batch, 512-query chunk):
           S^T[m,n] = kT.T @ qT (row-tiled K=64 matmul pairs),
           P = exp(S*scale) (no max subtraction needed: scores ~ N(0,1)),
           O^T_ext[65,n] = [V|1].T @ P^T accumulated over key tiles in PSUM
           (row 64 = softmax denominator). O and den are copied to SBUF so
           the PSUM bank frees immediately; normalization (one batched
           reciprocal over all 16 denominators + broadcast + multiply)
           happens at the end of the phase.
  AllToAll: each core sends its 2-head output columns for core j's token
           rows; receives full 1024 c_in x its 512 token rows (2MB/core).
  Phase C: out rows = attnT.T @ w_proj.T + b_proj for this core's 512 rows.
Host gathers: concat rows -> [4096, 1024] -> [2, 2048, 1024].
"""
import sys, os, types
import numpy as np

if "/opt/trn_rl_repo" not in sys.path and os.path.isdir("/opt/trn_rl_repo"):
    sys.path.append("/opt/trn_rl_repo")

import concourse.bass as bass
import concourse.mybir as mybir
import concourse.tile as tile
from concourse import bacc
from concourse.bass_utils import run_bass_kernel_spmd

F32 = mybir.dt.float32
F32R = mybir.dt.float32r
BF16 = mybir.dt.bfloat16
EXP = mybir.ActivationFunctionType.Exp

NCORES = 8
B, N, C, H, D = 2, 2048, 1024, 16, 64
NT = B * N          # 4096 flat tokens
KT = C // 128       # 8 contraction tiles of 128
QC = 512            # query-chunk width (one PSUM bank)
NU = NT // QC       # 8 (batch, qchunk) units == A2A shard count
NMT = N // 128      # 16 key tiles per batch
SCALE = 1.0 / 8.0   # 1/sqrt(D)
GRP = 2             # score banks per exp call (pipeline depth 3 over 6 banks)
XCH = 1024          # x load chunk width (4KB rows for efficient DMA)

TRACE = False       # test harness sets True to capture exec_time_ns
LAST_EXEC_NS = None

_NC = None


def _round_f32r(a: np.ndarray) -> np.ndarray:
    """Round-to-nearest-even to the fp32r (e8m10) grid, matching the PE."""
    u = np.ascontiguousarray(a, dtype=np.float32).view(np.uint32)
    lsb = (u >> np.uint32(13)) & np.uint32(1)
    r = (u + np.uint32(0x0FFF) + lsb) & np.uint32(0xFFFFE000)
    return r.view(np.float32)


def _install_ntff_hook():
    if "antenv.axon_hooks" in sys.modules:
        return
    try:
        import antenv
        from trn_agent_boot.trn_boot import _ntff_profile_via_ctypes
        mod = types.ModuleType("antenv.axon_hooks")
        _hook = [None]
        mod.set_axon_ntff_profile_hook = lambda h: _hook.__setitem__(0, h)
        mod.get_axon_ntff_profile_hook = lambda: _hook[0]
        sys.modules["antenv.axon_hooks"] = mod
        antenv.axon_hooks = mod
        mod.set_axon_ntff_profile_hook(
            _ntff_profile_via_ctypes("/opt/axon/libaxon_pjrt.so"))
    except Exception:
        pass


def _build():
    nc = bacc.Bacc("TRN2", target_bir_lowering=False, debug=False,
                   num_devices=NCORES)
    xT_ext = nc.dram_tensor("xT", [C, NT], BF16, kind="ExternalInput").ap()
    wT_ext = nc.dram_tensor("wT", [C, 384], BF16, kind="ExternalInput").ap()
    wpT_ext = nc.dram_tensor("wpT", [C, C], BF16, kind="ExternalInput").ap()
    bias_ext = nc.dram_tensor("bias", [1, C], F32, kind="ExternalInput").ap()
    idn_ext = nc.dram_tensor("idn", [128, 128], BF16, kind="ExternalInput").ap()
    out_ext = nc.dram_tensor("out", [NT // NCORES, C], F32,
                             kind="ExternalOutput").ap()
    a2a_in = nc.dram_tensor("a2a_in", [NCORES * 128, QC], BF16)
    a2a_out = nc.dram_tensor("a2a_out", [NCORES * 128, QC], BF16)

    xT_v = xT_ext.rearrange("(kt p) n -> p kt n", p=128)
    wT_v = wT_ext.rearrange("(kt p) f -> p kt f", p=128)
    wpT_v = wpT_ext.rearrange("(kt p) f -> p kt f", p=128)

    with tile.TileContext(nc) as tc:
        with (
            tc.tile_pool(name="const", bufs=1) as cpool,
            tc.tile_pool(name="resid", bufs=1) as rpool,
        ):
            wT_sb = cpool.tile([128, KT, 384], BF16)
            for kt in range(KT):
                nc.sync.dma_start(wT_sb[:, kt, :], wT_v[:, kt, :])
            idn = cpool.tile([128, 128], BF16)
            nc.sync.dma_start(idn[:], idn_ext[:])
            bias_sb = cpool.tile([1, C], F32)
            nc.sync.dma_start(bias_sb[:], bias_ext[:])
            bias_bc = cpool.tile([128, C], F32)
            nc.gpsimd.partition_broadcast(bias_bc[:], bias_sb[:])

            qT_sb = rpool.tile([128, NT], BF16)
            kT_sb = rpool.tile([128, NT], BF16)
            v_sb = rpool.tile([128, NT // 128, 130], BF16)
            nc.gpsimd.memset(v_sb[:, :, 64], 1.0)
            nc.gpsimd.memset(v_sb[:, :, 129], 1.0)
            # unnormalized attention outputs: block (u, h) lives at
            # [0:64, u*2+h, :] (base partition 0 so DVE ops can pair it
            # with broadcast tiles)
            stage = rpool.tile([64, 2 * NU, QC], F32)
            wp_sb = rpool.tile([128, KT, C], BF16)

            def qkv_groups(vpool, apsum, x_tiles, bat, psum_tag):
                """Yield one closure per QKV matmul-group (8 accumulating
                matmuls + PSUM evacuation; the v-feature groups also emit
                the PE transposes building V-natural)."""
                for nch2 in range(N // XCH):
                    x_t = x_tiles[bat * (N // XCH) + nch2]
                    for hw in range(XCH // QC):
                        ncol = bat * N + nch2 * XCH + hw * QC
                        for ft in range(3):
                            def emit(ncol=ncol, hw=hw, ft=ft, x_t=x_t):
                                xs = x_t[:, :, hw * QC:(hw + 1) * QC]
                                ps = apsum.tile([128, QC], F32, tag=psum_tag,
                                                name=f"qkv_{ncol}_{ft}")
                                for kt in range(KT):
                                    nc.tensor.matmul(
                                        ps[:],
                                        wT_sb[:, kt, ft * 128:(ft + 1) * 128],
                                        xs[:, kt, :],
                                        start=(kt == 0), stop=(kt == KT - 1))
                                if ft == 0:
                                    nc.vector.tensor_copy(
                                        out=qT_sb[:, ncol:ncol + QC],
                                        in_=ps[:])
                                elif ft == 1:
                                    nc.vector.tensor_copy(
                                        out=kT_sb[:, ncol:ncol + QC],
                                        in_=ps[:])
                                else:
                                    vt = vpool.tile([128, QC], BF16, tag="vt",
                                                    name=f"vt_{ncol}")
                                    nc.vector.tensor_copy(out=vt[:],
                                                          in_=ps[:])
                                    for t in range(4):
                                        mtg = ncol // 128 + t
                                        trp = apsum.tile(
                                            [128, 128], BF16, tag=psum_tag,
                                            name=f"tr_{mtg}")
                                        nc.tensor.transpose(
                                            trp[:],
                                            vt[:, t * 128:(t + 1) * 128],
                                            idn[:])
                                        nc.vector.tensor_copy(
                                            out=v_sb[:, mtg, 0:64],
                                            in_=trp[:, 0:64])
                                        nc.vector.tensor_copy(
                                            out=v_sb[:, mtg, 65:129],
                                            in_=trp[:, 64:128])
                            yield emit

            def attn_phase(spsum, opsum, ppool, denpool, rbpool, onpool,
                           bat, fillers=(), fill_every=3):
                fillers = list(fillers)
                fill_count = 0
                for uu in range(N // QC):
                    u = bat * (N // QC) + uu
                    qcol = u * QC
                    # heads sequential so a single PSUM accumulator bank
                    # suffices
                    units = ([(0, mt) for mt in range(NMT)]
                             + [(1, mt) for mt in range(NMT)])
                    o_cur = {}
                    for g0 in range(0, len(units), GRP):
                        g = units[g0:g0 + GRP]
                        s_t = spsum.tile([128, GRP, QC], F32, tag="s",
                                         name=f"s_{u}_{g0}")
                        for ui, (h, mt) in enumerate(g):
                            if mt == 0 and h not in o_cur:
                                o_cur[h] = opsum.tile(
                                    [65, QC], F32, tag="o",
                                    name=f"o_ps{h}_{u}")
                            nc.tensor.matmul(
                                s_t[:, ui, :],
                                kT_sb[h * 64:(h + 1) * 64,
                                      bat * N + mt * 128:
                                      bat * N + (mt + 1) * 128],
                                qT_sb[h * 64:(h + 1) * 64, qcol:qcol + QC],
                                start=True, stop=True)
                        p_t = ppool.tile([128, GRP, QC], BF16, tag="p",
                                         name=f"p_{u}_{g0}")
                        nc.scalar.activation(p_t[:, 0:len(g), :],
                                             s_t[:, 0:len(g), :],
                                             EXP, scale=SCALE)
                        for ui, (h, mt) in enumerate(g):
                            nc.tensor.matmul(
                                o_cur[h][:],
                                v_sb[:, bat * NMT + mt, h * 65:(h + 1) * 65],
                                p_t[:, ui, :],
                                start=(mt == 0), stop=(mt == NMT - 1))
                            if mt == NMT - 1:
                                o_ps = o_cur.pop(h)
                                nc.vector.tensor_copy(
                                    out=stage[:, u * 2 + h, :],
                                    in_=o_ps[0:64, :])
                                den = denpool.tile([1, QC], F32, tag="den",
                                                   name=f"den_{u}_{h}")
                                nc.vector.tensor_copy(out=den[:],
                                                      in_=o_ps[64:65, :])
                                rcp = denpool.tile([1, QC], F32, tag="rcp",
                                                   name=f"rcp_{u}_{h}")
                                nc.vector.reciprocal(rcp[:], den[:])
                                rb = rbpool.tile([64, QC], F32, tag="rb",
                                                 name=f"rb_{u}_{h}")
                                nc.gpsimd.partition_broadcast(rb[:], rcp[:])
                                o_n = onpool.tile([64, QC], BF16, tag="on",
                                                  name=f"on_{u}_{h}")
                                nc.vector.tensor_tensor(
                                    o_n[:], stage[:, u * 2 + h, :],
                                    rb[:], mybir.AluOpType.mult)
                                nc.sync.dma_start(
                                    a2a_in[u * 128 + h * 64:
                                           u * 128 + (h + 1) * 64, :],
                                    o_n[:])
                        fill_count += 1
                        if fillers and fill_count % fill_every == 0:
                            fillers.pop(0)()
                for f in fillers:
                    f()

            with (
                tc.tile_pool(name="xchunk", bufs=1) as xpool,
                tc.tile_pool(name="vtmp", bufs=2) as vpool,
                tc.tile_pool(name="pexp", bufs=4) as ppool,
                tc.tile_pool(name="denp", bufs=4) as denpool,
                tc.tile_pool(name="rbp", bufs=4) as rbpool,
                tc.tile_pool(name="onrm", bufs=4) as onpool,
            ):
                # batch-0 x chunks load first; batch-1 chunk DMAs overlap
                # batch-0 attention (no PSUM involved in a DMA)
                x_tiles = []
                for nch in range(NT // XCH):
                    x_t = xpool.tile([128, KT, XCH], BF16, tag=f"x{nch}",
                                     name=f"x_{nch}")
                    x_tiles.append(x_t)
                for nch in range(NT // XCH):
                    for kt in range(KT):
                        nc.sync.dma_start(
                            x_tiles[nch][:, kt, :],
                            xT_v[:, kt, nch * XCH:(nch + 1) * XCH])

                # batch-0 QKV up front (serial head of the pipeline)
                with (
                    tc.tile_pool(name="qkvps0", bufs=2,
                                 space="PSUM") as apsum0,
                ):
                    for emit in qkv_groups(vpool, apsum0, x_tiles, 0, "a0"):
                        emit()
                nc.sync.dma_start(wp_sb[:], wpT_v[:])
                # batch-0 attention with batch-1 QKV groups interleaved
                # into the PE/DVE idle (they use the single spare PSUM bank)
                with (
                    tc.tile_pool(name="sps0", bufs=3, space="PSUM") as spsum0,
                    tc.tile_pool(name="ops0", bufs=1, space="PSUM") as opsum0,
                    tc.tile_pool(name="fill", bufs=1, space="PSUM") as fpsum,
                ):
                    fillers = list(qkv_groups(vpool, fpsum, x_tiles, 1,
                                              "fill"))
                    attn_phase(spsum0, opsum0, ppool, denpool, rbpool,
                               onpool, 0, fillers=fillers, fill_every=3)
                with (
                    tc.tile_pool(name="sps1", bufs=3, space="PSUM") as spsum1,
                    tc.tile_pool(name="ops1", bufs=1, space="PSUM") as opsum1,
                ):
                    attn_phase(spsum1, opsum1, ppool, denpool, rbpool,
                               onpool, 1)

            nc.gpsimd.collective_compute(
                "AllToAll",
                mybir.AluOpType.bypass,
                replica_groups=[list(range(NCORES))],
                ins=[a2a_in[:]],
                outs=[a2a_out[:]],
            )

            # ---- Phase C: output projection for this core's 512 rows ----
            with (
                tc.tile_pool(name="plhs", bufs=1) as lpool,
                tc.tile_pool(name="pps", bufs=1, space="PSUM") as ppsum,
                tc.tile_pool(name="pout", bufs=2) as outpool,
            ):
                lhs = lpool.tile([128, KT, QC], BF16)
                for j in range(KT):
                    nc.sync.dma_start(lhs[:, j, :],
                                      a2a_out[j * 128:(j + 1) * 128, :])
                pp = {i: ppsum.tile([128, QC], F32, tag=f"pp{i}",
                                    name=f"pp_{i}")
                      for i in range(8)}
                for j in range(KT):
                    for mt in range(4):
                        for half in range(2):
                            nc.tensor.matmul(
                                pp[mt * 2 + half][:],
                                lhs[:, j, mt * 128:(mt + 1) * 128],
                                wp_sb[:, j, half * QC:(half + 1) * QC],
                                start=(j == 0), stop=(j == KT - 1))
                for mt in range(4):
                    for half in range(2):
                        ot = outpool.tile([128, QC], F32, tag="ot",
                                          name=f"ot_{mt}_{half}")
                        nc.vector.tensor_tensor(
                            ot[:], pp[mt * 2 + half][:],
                            bias_bc[:, half * QC:(half + 1) * QC],
                            mybir.AluOpType.add)
                        nc.sync.dma_start(
                            out_ext[mt * 128:(mt + 1) * 128,
                                    half * QC:(half + 1) * QC],
                            ot[:])
    nc.compile()
    return nc


def kernel(x, w_qkv, w_proj, b_proj):
    global _NC, LAST_EXEC_NS
    if _NC is None:
        _NC = _build()
    x = np.asarray(x, dtype=np.float32)
    w_qkv = np.asarray(w_qkv, dtype=np.float32)
    w_proj = np.asarray(w_proj, dtype=np.float32)
    b_proj = np.asarray(b_proj, dtype=np.float32)

    import ml_dtypes
    xT = np.ascontiguousarray(x.reshape(NT, C).T).astype(ml_dtypes.bfloat16)
    wpT = np.ascontiguousarray(w_proj.T).astype(ml_dtypes.bfloat16)
    bias = np.ascontiguousarray(b_proj.reshape(1, C))
    idn = np.eye(128, dtype=ml_dtypes.bfloat16)
    in_maps = []
    for c in range(NCORES):
        blk = slice(128 * c, 128 * (c + 1))
        wT = np.ascontiguousarray(
            np.concatenate([w_qkv[0:C][blk], w_qkv[C:2 * C][blk],
                            w_qkv[2 * C:3 * C][blk]], axis=0).T).astype(
                ml_dtypes.bfloat16)
        in_maps.append({"xT": xT, "wT": wT, "wpT": wpT, "bias": bias,
                        "idn": idn})

    if TRACE:
        _install_ntff_hook()
    res = run_bass_kernel_spmd(_NC, in_maps, core_ids=list(range(NCORES)),
                               trace=TRACE)
    LAST_EXEC_NS = res.exec_time_ns
    out = np.concatenate([res.results[i]["out"] for i in range(NCORES)],
                         axis=0)
    return np.ascontiguousarray(out.reshape(B, N, C).astype(np.float32))
